# revision 1
# baseline (speedup 1.0000x reference)
"""Trainium2 Bass kernel for causal MultiHeadAttention (B=4,S=2048,E=1024,H=16).

Sharding: 8 cores = (batch b, head-half) grid. Core c handles batch c//2 and
heads [8*(c%2), 8*(c%2)+8). Each core computes its 8 heads' attention and the
partial output projection (its 512 rows of Wo); the host sums the two partials
per batch and adds the bias (the 2-way "all-reduce" done at unshard time).

On-core dataflow (bf16 matmul operands, fp32 PSUM accumulation):
  P1: xT tiles via plain DMA (the host ships x pre-transposed in bf16)
  P2: QT/KT [dh, s] per head (zero-padded to 128 partitions so every weight
      load is a full FWL-eligible [128,128] tile), V natural [s, 8*dh] in one
      N=512 matmul per (s-tile, e-tile); V stored per head as [V | ones |
      zeros] 128-column tiles so the PV matmul also emits the softmax
      denominator row.
  P3: per (head-pair, q-chunk) unit: scoresT [t, sq] = K^T.Q, exp on ACT
      (scale=1/sqrt(dh) fused; no max-subtraction needed - scores are
      provably small for these 0.02-scale weights), causal mask on diagonal
      tile-pairs via host-precomputed 1024-wide masks, PV accumulation
      interleaved one unit behind the scores stream to keep PE fed, softmax
      denominators batched per head-pair: one (split) DVE reciprocal over
      [8, 512] rows, DRAM-bounce stride-0 DMA broadcast, one multiply.
  P4: output projection from outT [concat-head-dim, s] x Wo rows.
"""

import sys

if "/opt/trn_rl_repo" not in sys.path:
    sys.path.insert(0, "/opt/trn_rl_repo")

import numpy as np
from contextlib import ExitStack

B, S, E, H = 4, 2048, 1024, 16
DH = E // H          # 64
NCORES = 8
NH = 8               # local heads per core
HP = NH // 2         # head pairs
P = 128
NE = E // P          # 8 e-tiles
NT = S // P          # 16 s/t tiles
CH = 512
NCH = S // CH        # 4 q-chunks
SCALE = 1.0 / 8.0    # 1/sqrt(DH)

_CACHE = {}


def _build_nc():
    import concourse.mybir as mybir
    import concourse.tile as tile
    import concourse.bass as bass
    from concourse import bacc

    f32 = mybir.dt.float32
    bf16 = mybir.dt.bfloat16
    Exp = mybir.ActivationFunctionType.Exp
    PSUM = bass.MemorySpace.PSUM

    nc = bacc.Bacc(None)
    x_d = nc.dram_tensor("x", [E, S], bf16, kind="ExternalInput")  # pre-transposed
    wq_d = nc.dram_tensor("wq", [E, NH * DH], bf16, kind="ExternalInput")
    wk_d = nc.dram_tensor("wk", [E, NH * DH], bf16, kind="ExternalInput")
    wv_d = nc.dram_tensor("wv", [E, NH * DH], bf16, kind="ExternalInput")
    wo_d = nc.dram_tensor("wo", [NH * DH, E], bf16, kind="ExternalInput")
    mask_d = nc.dram_tensor("mask", [P, 2, 2 * CH], bf16, kind="ExternalInput")
    zz_d = nc.dram_tensor("zz", [P, NT * NH * P], bf16, kind="ExternalInput")
    out_d = nc.dram_tensor("out", [S, E], f32, kind="ExternalOutput")

    with ExitStack() as ctx:
        tc = ctx.enter_context(tile.TileContext(nc))
        persist = ctx.enter_context(tc.tile_pool(name="persist", bufs=1))
        # per-head layouts, zero-padded to 128 partitions / 128 columns so
        # every matmul weight load is a full FWL-eligible [128,128] tile
        qt = persist.tile([P, NH, S], bf16)           # rows 64:128 zero
        kt = persist.tile([P, NH, S], bf16)
        vf = persist.tile([P, NT, NH, P], bf16)       # V | ones | zeros
        msk = persist.tile([P, 2, 2 * CH], bf16)
        nc.sync.dma_start(out=msk, in_=mask_d[:])
        zq = qt[DH:P, :, :].rearrange("p a b -> p (a b)")
        zk = kt[DH:P, :, :].rearrange("p a b -> p (a b)")
        zv = vf.rearrange("p a b c -> p (a b c)")
        nc.scalar.dma_start(out=zv, in_=zz_d[:, :])
        nc.scalar.dma_start(out=zq, in_=zz_d[0:DH, :])
        nc.scalar.dma_start(out=zk, in_=zz_d[0:DH, :])

        with ExitStack() as pha:
            xtp = pha.enter_context(tc.tile_pool(name="xtp", bufs=1))
            wvp = pha.enter_context(tc.tile_pool(name="wvp", bufs=1))
            wqk = pha.enter_context(tc.tile_pool(name="wqk", bufs=1))

            # wv first (needed for the first matmuls), then the x transposes
            # on the SP HWDGE queue; wq/wk/mask ride the ACT HWDGE queue in
            # parallel (they are needed only later).
            ones = wvp.tile([P, NH], bf16)
            nc.vector.memset(ones, 1.0)
            # interleave per-e-tile wv and xT loads so the first V-projection
            # accumulation chain can start as soon as (wv0, xt0) land
            wvs, xts = [], []
            for et in range(NE):
                wv = wvp.tile([P, NH * DH], bf16, tag=f"wv{et}", name="wv")
                nc.sync.dma_start(out=wv, in_=wv_d[et * P:(et + 1) * P, :])
                wvs.append(wv)
                xt = xtp.tile([P, S], bf16, tag=f"xt{et}", name="xt")
                nc.sync.dma_start(out=xt, in_=x_d[et * P:(et + 1) * P, :])
                xts.append(xt)

            wts = {}
            for hp in range(HP):
                for wi, wd in enumerate((wq_d, wk_d)):
                    wt = wqk.tile([P, NE, P], bf16, tag=f"wt{hp}{wi}",
                                  name="wt")
                    for et in range(NE):
                        nc.scalar.dma_start(
                            out=wt[:, et, :],
                            in_=wd[et * P:(et + 1) * P, hp * P:(hp + 1) * P])
                    wts[(hp, wi)] = wt

            # ---- P2a: V natural (all 8 heads per matmul) ----
            with ExitStack() as p2a:
                vps = p2a.enter_context(tc.tile_pool(name="vps", bufs=6, space=PSUM))
                for st in range(NT):
                    ps = vps.tile([P, NH * DH], f32)
                    for et in range(NE):
                        nc.tensor.matmul(
                            ps, xts[et][:, st * P:(st + 1) * P], wvs[et],
                            start=(et == 0), stop=(et == NE - 1))
                    nc.vector.tensor_copy(
                        out=vf[:, st, :, 0:DH],
                        in_=ps.rearrange("p (h d) -> p h d", h=NH))
                    nc.vector.tensor_copy(
                        out=vf[:, st, :, DH:DH + 1], in_=ones.unsqueeze(2))

            # ---- P2b: QT / KT (2 heads per matmul, split into per-head
            #      zero-padded layout on copy-out) ----
            with ExitStack() as p2b:
                qks = p2b.enter_context(tc.tile_pool(name="qks", bufs=6, space=PSUM))
                for hp in range(HP):
                    for wi, dst in ((0, qt), (1, kt)):
                        wt = wts[(hp, wi)]
                        for chk in range(NCH):
                            ps = qks.tile([P, CH], f32)
                            for et in range(NE):
                                nc.tensor.matmul(
                                    ps, wt[:, et, :],
                                    xts[et][:, chk * CH:(chk + 1) * CH],
                                    start=(et == 0), stop=(et == NE - 1))
                            cs = slice(chk * CH, (chk + 1) * CH)
                            nc.vector.tensor_copy(
                                out=dst[0:DH, 2 * hp, cs], in_=ps[0:DH, :])
                            nc.vector.tensor_copy(
                                out=dst[0:DH, 2 * hp + 1, cs], in_=ps[DH:P, :])

        # xT freed here
        with ExitStack() as phb:
            otp = phb.enter_context(tc.tile_pool(name="otp", bufs=1))
            outTs = [otp.tile([P, S], bf16, tag=f"outT{i}", name="outT")
                     for i in range(HP)]

            # ---- P3: attention; PV pipelined one (hp,chunk) unit behind ----
            with ExitStack() as p3:
                ptp = p3.enter_context(tc.tile_pool(name="ptp", bufs=24))
                pvo = p3.enter_context(tc.tile_pool(name="pvo", bufs=8))
                dnp = p3.enter_context(tc.tile_pool(name="dnp", bufs=8))
                dn8 = p3.enter_context(tc.tile_pool(name="dn8", bufs=2))
                bcp = p3.enter_context(tc.tile_pool(name="bcp", bufs=6))
                drp = p3.enter_context(tc.tile_pool(name="drp", bufs=2,
                                                    space="DRAM"))
                scp = p3.enter_context(tc.tile_pool(name="scp", bufs=3, space=PSUM))
                pvp = p3.enter_context(tc.tile_pool(name="pvp", bufs=2, space=PSUM))

                hp_dens = {}     # hp -> dens tile [8, CH]
                hp_outs = {}     # hp -> list of (chk, po tile)

                def emit_unit(hp, chk, pending):
                    """Scores+exp+mask for (hp,chk), with the previous unit's
                    PV matmuls interleaved into the PE stream so PE can fill
                    the ACT-throttled gaps between score pairs."""
                    ntv = 4 * chk + 4      # valid t-tiles
                    nprs = ntv // 2
                    pts = {0: [], 1: []}
                    pv_mms = []
                    if pending is not None:
                        phl, pchk, ppts = pending
                        pntv = 4 * pchk + 4
                        pvs = {}
                        for h in range(2):
                            pvs[h] = pvp.tile([P, CH], f32, tag="pv",
                                              name="pv")
                        for h in range(2):
                            for tt in range(pntv):
                                pv_mms.append((phl, pchk, ppts, pvs, h, tt,
                                               pntv))
                    done = 0
                    for pr in range(nprs):
                        sps = {}
                        for j in range(2):
                            tt = 2 * pr + j
                            for h in range(2):
                                hl = 2 * hp + h
                                if h not in sps:
                                    sps[h] = scp.tile(
                                        [P, 2 * CH], f32, tag="sp", name="sp")
                                nc.tensor.matmul(
                                    sps[h][:, j * CH:(j + 1) * CH],
                                    kt[:, hl, tt * P:(tt + 1) * P],
                                    qt[:, hl, chk * CH:(chk + 1) * CH],
                                    start=True, stop=True)
                        for h in range(2):
                            pt = ptp.tile([P, 2 * CH], bf16, tag="pt", name="pt")
                            nc.scalar.activation(
                                out=pt, in_=sps[h], func=Exp, scale=SCALE)
                            jdx = pr - 2 * chk   # 0/1 for the diagonal pairs
                            if jdx >= 0:
                                nc.vector.tensor_mul(pt, pt, msk[:, jdx, :])
                            pts[h].append(pt)
                        want = (pr + 1) * len(pv_mms) // nprs
                        while done < want:
                            emit_pv_mm(*pv_mms[done])
                            done += 1
                    while done < len(pv_mms):
                        emit_pv_mm(*pv_mms[done])
                        done += 1
                    if pending is not None:
                        emit_pv_tail(pending[0], pending[1], pvs)
                    return pts

                def emit_pv_mm(hp, chk, pts, pvs, h, tt, ntv):
                    nc.tensor.matmul(
                        pvs[h],
                        vf[:, tt, 2 * hp + h, :],
                        pts[h][tt // 2][:, (tt % 2) * CH:(tt % 2 + 1) * CH],
                        start=(tt == 0), stop=(tt == ntv - 1),
                        skip_group_check=True)

                def emit_pv_tail(hp, chk, pvs):
                    if hp not in hp_dens:
                        hp_dens[hp] = dn8.tile([2 * NCH, CH], f32, tag="dens",
                                               name="dens")
                        hp_outs[hp] = []
                    po = pvo.tile([P, CH], bf16, tag="po", name="po")
                    for h in range(2):
                        pv = pvs[h]
                        # numerators -> po rows [64h, 64h+64); denom -> dens row
                        nc.vector.tensor_copy(
                            out=po[h * DH:(h + 1) * DH, :], in_=pv[0:DH, :])
                        den = dnp.tile([1, CH], f32, tag="den", name="den")
                        nc.vector.tensor_copy(out=den, in_=pv[DH:DH + 1, :])
                        nc.sync.dma_start(
                            out=hp_dens[hp][2 * chk + h:2 * chk + h + 1, :],
                            in_=den)
                    hp_outs[hp].append((chk, po))
                    if chk == NCH - 1:
                        fin_q.append(hp)

                def emit_finalize(hp):
                    # one reciprocal for all 8 denominator rows, then
                    # broadcast each row via DRAM-source stride-0 DMA
                    if True:
                        dens = hp_dens.pop(hp)
                        for k in range(4):
                            ks = slice(k * P, (k + 1) * P)
                            nc.vector.reciprocal(
                                out=dens[:, ks], in_=dens[:, ks])
                        dd = drp.tile([2 * NCH, CH], f32, tag="dd", name="dd")
                        nc.sync.dma_start(out=dd, in_=dens)
                        for ck, po_t in hp_outs.pop(hp):
                            bc = bcp.tile([P, CH], f32, tag="bc", name="bc")
                            for h in range(2):
                                row = dd[2 * ck + h:2 * ck + h + 1, :]
                                src = bass.AP(
                                    tensor=row.tensor, offset=row.offset,
                                    ap=[[0, DH]] + list(row.ap[1:]))
                                nc.sync.dma_start(
                                    out=bc[h * DH:(h + 1) * DH, :], in_=src)
                            cs = slice(ck * CH, (ck + 1) * CH)
                            nc.gpsimd.tensor_mul(
                                outTs[hp][:, cs], po_t, bc)

                from collections import deque
                pend_q = deque()
                fin_q = []
                for hp in range(HP):
                    for chk in range(NCH):
                        pending = (pend_q.popleft()
                                   if len(pend_q) >= 2 else None)
                        pts = emit_unit(hp, chk, pending)
                        if fin_q:
                            emit_finalize(fin_q.pop(0))
                        pend_q.append((hp, chk, pts))
                # flush remaining PVs
                while pend_q:
                    fhp, fchk, fpts = pend_q.popleft()
                    fpvs = {h: pvp.tile([P, CH], f32, tag="pv", name="pv")
                            for h in range(2)}
                    fntv = 4 * fchk + 4
                    for h in range(2):
                        for tt in range(fntv):
                            emit_pv_mm(fhp, fchk, fpts, fpvs, h, tt, fntv)
                    emit_pv_tail(fhp, fchk, fpvs)
                while fin_q:
                    emit_finalize(fin_q.pop(0))

            # ---- P4: output projection (partial: local 512 rows of Wo) ----
            with ExitStack() as p4:
                wop = p4.enter_context(tc.tile_pool(name="wop", bufs=2))
                osb = p4.enter_context(tc.tile_pool(name="osb", bufs=4))
                ops = p4.enter_context(tc.tile_pool(name="ops", bufs=4, space=PSUM))
                for ech in range(E // CH):
                    wt2 = wop.tile([P, HP, CH], bf16, tag="wt2")
                    for hp in range(HP):
                        nc.sync.dma_start(
                            out=wt2[:, hp, :],
                            in_=wo_d[hp * P:(hp + 1) * P, ech * CH:(ech + 1) * CH])
                    for st in range(NT):
                        ps = ops.tile([P, CH], f32)
                        for hp in range(HP):
                            nc.tensor.matmul(
                                ps, outTs[hp][:, st * P:(st + 1) * P],
                                wt2[:, hp, :],
                                start=(hp == 0), stop=(hp == HP - 1))
                        ob = osb.tile([P, CH], f32)
                        nc.vector.tensor_copy(out=ob, in_=ps)
                        nc.sync.dma_start(
                            out=out_d[st * P:(st + 1) * P, ech * CH:(ech + 1) * CH],
                            in_=ob)

    nc.finalize()
    return nc


def _get_nc():
    if "nc" not in _CACHE:
        _CACHE["nc"] = _build_nc()
    return _CACHE["nc"]


def _make_in_maps(x, Wq, Wk, Wv, Wo):
    import ml_dtypes

    bf = ml_dtypes.bfloat16
    # mask[p, jdx, 512*j + f] = 1 iff p <= f - 128*(2*jdx + j): causal mask for
    # the diagonal t-tile pair jdx of any q-chunk (tt_rel = 2*jdx + j).
    pcol = np.arange(P)[:, None]
    frow = np.arange(CH)[None, :]
    blocks = [(pcol <= frow - 128 * r) for r in range(4)]
    mask = np.stack(
        [np.concatenate(blocks[0:2], axis=1),
         np.concatenate(blocks[2:4], axis=1)], axis=1).astype(bf)
    zz = np.zeros((P, NT * NH * P), dtype=bf)
    in_maps = []
    for c in range(NCORES):
        b, half = divmod(c, 2)
        hs = slice(half * NH, (half + 1) * NH)
        in_maps.append({
            "x": np.ascontiguousarray(x[b].T.astype(bf)),
            "wq": np.ascontiguousarray(
                Wq[hs].transpose(1, 0, 2).reshape(E, NH * DH).astype(bf)),
            "wk": np.ascontiguousarray(
                Wk[hs].transpose(1, 0, 2).reshape(E, NH * DH).astype(bf)),
            "wv": np.ascontiguousarray(
                Wv[hs].transpose(1, 0, 2).reshape(E, NH * DH).astype(bf)),
            "wo": np.ascontiguousarray(
                Wo[half * NH * DH:(half + 1) * NH * DH].astype(bf)),
            "mask": mask,
            "zz": zz,
        })
    return in_maps


def _ensure_ntff_hook():
    """Register the axon NTFF profile hook under antenv.axon_hooks.

    The agent image's antenv lacks the axon_hooks module, so
    run_bass_kernel_spmd(trace=True) would silently skip profiling.
    Recreate the module in sys.modules using trn_agent_boot's ctypes hook.
    """
    import types
    try:
        import antenv.axon_hooks  # noqa: F401
        return
    except ImportError:
        pass
    try:
        from trn_agent_boot.trn_boot import _ntff_profile_via_ctypes
        hook = _ntff_profile_via_ctypes("/opt/axon/libaxon_pjrt.so")
    except Exception:
        hook = None
    mod = types.ModuleType("antenv.axon_hooks")
    mod.get_axon_ntff_profile_hook = lambda: hook
    mod.set_axon_ntff_profile_hook = lambda h: None
    sys.modules["antenv.axon_hooks"] = mod


def _run(inputs, trace=False):
    from concourse.bass_utils import run_bass_kernel_spmd

    if trace:
        _ensure_ntff_hook()

    x = np.asarray(inputs["x"], dtype=np.float32)
    Wq = np.asarray(inputs["Wq"], dtype=np.float32)
    Wk = np.asarray(inputs["Wk"], dtype=np.float32)
    Wv = np.asarray(inputs["Wv"], dtype=np.float32)
    Wo = np.asarray(inputs["Wo"], dtype=np.float32)
    bo = np.asarray(inputs["bo"], dtype=np.float32)

    nc = _get_nc()
    in_maps = _make_in_maps(x, Wq, Wk, Wv, Wo)
    res = run_bass_kernel_spmd(nc, in_maps, list(range(NCORES)), trace=trace)
    out = np.empty((B, S, E), dtype=np.float32)
    for b in range(B):
        out[b] = res.results[2 * b]["out"] + res.results[2 * b + 1]["out"] + bo
    return out, res


def kernel(**inputs):
    out, _ = _run(inputs, trace=False)
    return out



# revision 2
# speedup vs baseline: 1.0523x; 1.0523x over previous
"""Trainium2 Bass kernel v2 for causal MultiHeadAttention (B=4,S=2048,E=1024,H=16).

Sharding: 8 cores = (batch, head-half) grid as v1. Core c: batch c//2, heads
[8*(c%2), 8*(c%2)+8). Host sums the two half-projections per batch + bias.

v2 changes vs v1 (all bf16 — fp8 fails the 2e-2 gate, measured 2.8-6e-2):
  - Scores via PE row-group concurrency: Q/K kept packed [128 = 2 heads x 64dh]
    (no zero-padding, no split copyouts). Per t-tile the two heads' K=64
    matmuls alternate base partitions 0/64 -> concurrent in the PE array
    (measured 73.3ns per [128,256] matmul vs 229ns sequential).
  - CH=256 q-chunks: tighter causal coverage (56.25% vs 62.5%) -> less exp
    and PV work; exp batched over 2 t-tile pairs [128,1024] per head.
  - Chunk-major unit order with a work queue: P2 projection chains, PV
    chains, and P4 out-projection chains are drained into the ACT-throttled
    P3 stream, so the PE never idles between phases and P4 has no tail.
"""

import sys

if "/opt/trn_rl_repo" not in sys.path:
    sys.path.insert(0, "/opt/trn_rl_repo")

import numpy as np
from collections import deque
from contextlib import ExitStack

B, S, E, H = 4, 2048, 1024, 16
DH = E // H          # 64
NCORES = 8
NH = 8               # local heads per core
HP = NH // 2         # head pairs
P = 128
NE = E // P          # 8 e-tiles
NT = S // P          # 16 t-tiles
CH = 256             # q-chunk width
NCH = S // CH        # 8 chunks
SC = S // 512        # 4 sq-supertiles for P2
SCALE = 1.0 / 8.0    # 1/sqrt(DH)

_CACHE = {}


def _build_nc():
    import concourse.mybir as mybir
    import concourse.tile as tile
    import concourse.bass as bass
    from concourse import bacc

    f32 = mybir.dt.float32
    bf16 = mybir.dt.bfloat16
    Exp = mybir.ActivationFunctionType.Exp
    PSUM = bass.MemorySpace.PSUM

    nc = bacc.Bacc(None)
    x_d = nc.dram_tensor("x", [E, S], bf16, kind="ExternalInput")  # pre-transposed
    wq_d = nc.dram_tensor("wq", [E, NH * DH], bf16, kind="ExternalInput")
    wk_d = nc.dram_tensor("wk", [E, NH * DH], bf16, kind="ExternalInput")
    wv_d = nc.dram_tensor("wv", [E, NH * DH], bf16, kind="ExternalInput")
    wo_d = nc.dram_tensor("wo", [NH * DH, E], bf16, kind="ExternalInput")
    mask_d = nc.dram_tensor("mask", [P, 2 * CH], bf16, kind="ExternalInput")
    out_d = nc.dram_tensor("out", [S, E], f32, kind="ExternalOutput")

    with ExitStack() as ctx:
        tc = ctx.enter_context(tile.TileContext(nc))
        persist = ctx.enter_context(tc.tile_pool(name="persist", bufs=1))
        # packed per-head-pair layouts: partitions 0:64 head 2hp, 64:128 head
        # 2hp+1 (dh dim); scores use quadrant matmuls straight off these.
        qt = persist.tile([P, HP, S], bf16)
        kt = persist.tile([P, HP, S], bf16)
        # V natural per (t-tile, head): [64 v-cols | ones | 63 garbage]
        vf = persist.tile([P, NT, NH, P], bf16)
        outT = persist.tile([P, HP, S], bf16)
        wo = persist.tile([P, HP, E], bf16)
        msk = persist.tile([P, 2 * CH], bf16)

        p2 = ctx.enter_context(tc.tile_pool(name="p2", bufs=1))
        xt = p2.tile([P, NE, S], bf16)
        wqs = p2.tile([P, NE, NH * DH], bf16)
        wks = p2.tile([P, NE, NH * DH], bf16)
        wvs = p2.tile([P, NE, NH * DH], bf16)

        # DMAs: x/wv on the SP queue (feeds the first chains); wq/wk/mask/wo
        # on the ACT queue in parallel.
        # Spread the startup-critical loads (x chunk 0 + wq) across four DMA
        # queues so the first QK chains can start ~5us in; wv/wk follow on
        # the same queue pairs, then the rest of x.
        for sc in range(SC):
            cs = slice(sc * 512, (sc + 1) * 512)
            for et in range(NE):
                nc.sync.dma_start(out=xt[:, et, cs],
                                  in_=x_d[et * P:(et + 1) * P, cs])
            if sc == 0:
                for et in range(NE):
                    nc.sync.dma_start(out=wvs[:, et, :],
                                      in_=wv_d[et * P:(et + 1) * P, :])
        for et in range(NE):
            nc.scalar.dma_start(out=wqs[:, et, :], in_=wq_d[et * P:(et + 1) * P, :])
            nc.scalar.dma_start(out=wks[:, et, :], in_=wk_d[et * P:(et + 1) * P, :])
        nc.scalar.dma_start(out=msk, in_=mask_d[:])
        for hp in range(HP):
            nc.scalar.dma_start(out=wo[:, hp, :],
                                in_=wo_d[hp * P:(hp + 1) * P, :])
        nc.vector.memset(vf[:, :, :, DH:DH + 1], 1.0)

        prj = ctx.enter_context(tc.tile_pool(name="prj", bufs=2, space=PSUM))
        scp = ctx.enter_context(tc.tile_pool(name="scp", bufs=2, space=PSUM))
        pvp = ctx.enter_context(tc.tile_pool(name="pvp", bufs=2, space=PSUM))
        ptp = ctx.enter_context(tc.tile_pool(name="ptp", bufs=16))
        pop = ctx.enter_context(tc.tile_pool(name="pop", bufs=10))
        dnp = ctx.enter_context(tc.tile_pool(name="dnp", bufs=8))
        dcp = ctx.enter_context(tc.tile_pool(name="dcp", bufs=2))
        bcp = ctx.enter_context(tc.tile_pool(name="bcp", bufs=6))
        obp = ctx.enter_context(tc.tile_pool(name="obp", bufs=3))
        drp = ctx.enter_context(tc.tile_pool(name="drp", bufs=2, space="DRAM"))

        # ---------- emit helpers ----------
        def emit_v_chain(st):
            ps = prj.tile([P, 512], f32, tag="prj", name="ps")
            for et in range(NE):
                nc.tensor.matmul(ps, xt[:, et, st * P:(st + 1) * P],
                                 wvs[:, et, :], start=(et == 0),
                                 stop=(et == NE - 1), skip_group_check=True)
            nc.vector.tensor_copy(
                out=vf[:, st, :, 0:DH],
                in_=ps.rearrange("p (h d) -> p h d", h=NH))

        def emit_qk_chain(hp, wi, sc):
            w = wqs if wi == 0 else wks
            dst = qt if wi == 0 else kt
            ps = prj.tile([P, 512], f32, tag="prj", name="ps")
            cs = slice(sc * 512, (sc + 1) * 512)
            for et in range(NE):
                nc.tensor.matmul(ps, w[:, et, hp * P:(hp + 1) * P],
                                 xt[:, et, cs], start=(et == 0),
                                 stop=(et == NE - 1), skip_group_check=True)
            nc.vector.tensor_copy(out=dst[:, hp, cs], in_=ps)

        workq = deque()
        reserve = {"on": False}  # hold back P4 items as tail filler

        def drain_budget(ns):
            skipped = deque()
            while workq and ns > 0:
                it = workq.popleft()
                if (reserve["on"] and it["kind"] == "p4"
                        and len(skipped) < 6):
                    skipped.append(it)
                    continue
                it["fn"]()
                ns -= it["cost"]
            workq.extendleft(reversed(skipped))

        def drain_pred(pred):
            keep = deque()
            while workq:
                it = workq.popleft()
                if pred(it):
                    it["fn"]()
                else:
                    keep.append(it)
            workq.extend(keep)

        # ---------- P3 scores unit ----------
        # One burst = up to 4 t-tiles x 2 heads (A/B alternating per t-tile
        # so head-B LDWEIGHTS overlap head-A matmuls), emitted back-to-back
        # with NO intervening queue work; queue drains happen only between
        # bursts, after the exps are emitted.
        def emit_unit(chk, hp):
            ntv = 2 * (chk + 1)
            cs = slice(chk * CH, (chk + 1) * CH)
            pts = {0: [], 1: []}
            for g in range((ntv + 3) // 4):
                tts = list(range(4 * g, min(4 * g + 4, ntv)))
                sps = {0: scp.tile([P, 4, CH], f32, tag="sp", name="sp"),
                       1: scp.tile([P, 4, CH], f32, tag="sp", name="sp")}
                for tt in tts:
                    ts = slice(tt * P, (tt + 1) * P)
                    for h in range(2):
                        b = 64 * h
                        nc.tensor.matmul(
                            sps[h][:, tt - 4 * g, :], kt[b:b + 64, hp, ts],
                            qt[b:b + 64, hp, cs], start=True, stop=True)
                n = len(tts)
                for h in range(2):
                    pt = ptp.tile([P, 4, CH], bf16, tag="pt", name="pt")
                    nc.scalar.activation(
                        out=pt[:, 0:n, :].rearrange("p a b -> p (a b)"),
                        in_=sps[h][:, 0:n, :].rearrange("p a b -> p (a b)"),
                        func=Exp, scale=SCALE)
                    j0 = 2 * chk - 4 * g  # diagonal pair's local slot
                    if 0 <= j0 < 4:
                        nc.vector.tensor_mul(
                            pt[:, j0:j0 + 2, :].rearrange("p a b -> p (a b)"),
                            pt[:, j0:j0 + 2, :].rearrange("p a b -> p (a b)"),
                            msk)
                    pts[h].append(pt)
                drain_budget(2500)
            return pts

        po_units = {}    # chk -> {hp: po tile}
        dchks = {}       # chk -> dens tile

        def emit_pv(chk, hp, pts):
            ntv = 2 * (chk + 1)
            if chk not in dchks:
                dchks[chk] = dcp.tile([NH, CH], f32, tag="dc", name="dc")
            po = pop.tile([P, CH], bf16, tag="po", name="po")
            for h in range(2):
                hl = 2 * hp + h
                pv = pvp.tile([P, 512], f32, tag="pv", name="pv")
                for tt in range(ntv):
                    nc.tensor.matmul(
                        pv[:, 0:CH], vf[:, tt, hl, :],
                        pts[h][tt // 4][:, tt % 4, :],
                        start=(tt == 0), stop=(tt == ntv - 1),
                        skip_group_check=True)
                nc.vector.tensor_copy(out=po[h * DH:(h + 1) * DH, :],
                                      in_=pv[0:DH, 0:CH])
                den = dnp.tile([1, CH], f32, tag="den", name="den")
                nc.vector.tensor_copy(out=den, in_=pv[DH:DH + 1, 0:CH])
                nc.sync.dma_start(
                    out=dchks[chk][2 * hp + h:2 * hp + h + 1, :], in_=den)
            po_units.setdefault(chk, {})[hp] = po

        def emit_finalize(chk):
            dens = dchks.pop(chk)
            for k in range(2):
                ks = slice(k * P, (k + 1) * P)
                nc.vector.reciprocal(out=dens[:, ks], in_=dens[:, ks])
            dd = drp.tile([NH, CH], f32, tag="dd", name="dd")
            nc.sync.dma_start(out=dd, in_=dens)
            pos = po_units.pop(chk)
            cs = slice(chk * CH, (chk + 1) * CH)
            for hp in range(HP):
                bc = bcp.tile([P, CH], f32, tag="bc", name="bc")
                for h in range(2):
                    row = dd[2 * hp + h:2 * hp + h + 1, :]
                    src = bass.AP(tensor=row.tensor, offset=row.offset,
                                  ap=[[0, DH]] + list(row.ap[1:]))
                    nc.sync.dma_start(out=bc[h * DH:(h + 1) * DH, :], in_=src)
                nc.gpsimd.tensor_mul(outT[:, hp, cs], pos[hp], bc)

        def emit_p4(chk, st, ech):
            ps = prj.tile([P, 512], f32, tag="prj", name="ps")
            es = slice(ech * 512, (ech + 1) * 512)
            for hp in range(HP):
                nc.tensor.matmul(ps, outT[:, hp, st * P:(st + 1) * P],
                                 wo[:, hp, es], start=(hp == 0),
                                 stop=(hp == HP - 1), skip_group_check=True)
            ob = obp.tile([P, 512], f32, tag="ob", name="ob")
            nc.vector.tensor_copy(out=ob, in_=ps)
            nc.sync.dma_start(out=out_d[st * P:(st + 1) * P, es], in_=ob)

        # ---------- emission schedule ----------
        # P2 sc=0 inline: QK first (unblocks chunk-0 scores), then V st 0-3.
        for hp in range(HP):
            for wi in range(2):
                emit_qk_chain(hp, wi, 0)
        for st in range(4):
            emit_v_chain(st)
        # queue the rest of P2
        for sc in range(1, SC):
            for hp in range(HP):
                for wi in range(2):
                    workq.append({"kind": "p2", "key": sc, "cost": 2200,
                                  "fn": (lambda hp=hp, wi=wi, sc=sc:
                                         emit_qk_chain(hp, wi, sc))})
            for st in range(4 * sc, 4 * sc + 4):
                workq.append({"kind": "p2", "key": sc, "cost": 2200,
                              "fn": (lambda st=st: emit_v_chain(st))})

        fin_pending = None
        prev_unit = None
        for pos, chk in enumerate(range(NCH)):
            if pos >= 6:
                reserve["on"] = True
            screq = (2 * chk + 1) // 4
            drain_pred(lambda it, r=screq: it["kind"] == "p2" and it["key"] <= r)
            for hp in range(HP):
                pts = emit_unit(chk, hp)
                if prev_unit is not None:
                    emit_pv(*prev_unit)     # lag-1: its exps are long done
                prev_unit = (chk, hp, pts)
                if fin_pending is not None:
                    fc = fin_pending
                    fin_pending = None
                    emit_finalize(fc)
                    for st in (2 * fc, 2 * fc + 1):
                        for ech in range(2):
                            workq.append({"kind": "p4", "key": fc, "cost": 1100,
                                          "fn": (lambda fc=fc, st=st, ech=ech:
                                                 emit_p4(fc, st, ech))})
            fin_pending = chk
        # tail: spend the reserved P4 work first (it fills the PE while ACT
        # chews through the last chunk's exp backlog), then the final
        # PV/finalize/P4 cascade.
        reserve["on"] = False
        drain_pred(lambda it: it["kind"] == "p4")
        emit_pv(*prev_unit)
        drain_pred(lambda it: it["kind"] != "p4")
        emit_finalize(fin_pending)
        for st in (2 * fin_pending, 2 * fin_pending + 1):
            for ech in range(2):
                workq.append({"kind": "p4", "key": fin_pending, "cost": 1100,
                              "fn": (lambda fc=fin_pending, st=st, ech=ech:
                                     emit_p4(fc, st, ech))})
        drain_pred(lambda it: True)

    nc.finalize()
    return nc


def _get_nc():
    if "nc" not in _CACHE:
        _CACHE["nc"] = _build_nc()
    return _CACHE["nc"]


def _make_in_maps(x, Wq, Wk, Wv, Wo):
    import ml_dtypes

    bf = ml_dtypes.bfloat16
    # mask[p, i*CH + n] = 1 iff 128*i + p <= n  (diagonal t-tile pair of any
    # CH=256 chunk; t_local = 128*i + p, sq_local = n)
    pcol = np.arange(P)[:, None]
    nrow = np.arange(CH)[None, :]
    mask = np.concatenate([(pcol <= nrow), (pcol + 128 <= nrow)],
                          axis=1).astype(bf)
    in_maps = []
    for c in range(NCORES):
        b, half = divmod(c, 2)
        hs = slice(half * NH, (half + 1) * NH)
        in_maps.append({
            "x": np.ascontiguousarray(x[b].T.astype(bf)),
            "wq": np.ascontiguousarray(
                Wq[hs].transpose(1, 0, 2).reshape(E, NH * DH).astype(bf)),
            "wk": np.ascontiguousarray(
                Wk[hs].transpose(1, 0, 2).reshape(E, NH * DH).astype(bf)),
            "wv": np.ascontiguousarray(
                Wv[hs].transpose(1, 0, 2).reshape(E, NH * DH).astype(bf)),
            "wo": np.ascontiguousarray(
                Wo[half * NH * DH:(half + 1) * NH * DH].astype(bf)),
            "mask": mask,
        })
    return in_maps


def _ensure_ntff_hook():
    import types
    try:
        import antenv.axon_hooks  # noqa: F401
        return
    except ImportError:
        pass
    try:
        from trn_agent_boot.trn_boot import _ntff_profile_via_ctypes
        hook = _ntff_profile_via_ctypes("/opt/axon/libaxon_pjrt.so")
    except Exception:
        hook = None
    mod = types.ModuleType("antenv.axon_hooks")
    mod.get_axon_ntff_profile_hook = lambda: hook
    mod.set_axon_ntff_profile_hook = lambda h: None
    sys.modules["antenv.axon_hooks"] = mod


def _run(inputs, trace=False):
    from concourse.bass_utils import run_bass_kernel_spmd

    if trace:
        _ensure_ntff_hook()

    x = np.asarray(inputs["x"], dtype=np.float32)
    Wq = np.asarray(inputs["Wq"], dtype=np.float32)
    Wk = np.asarray(inputs["Wk"], dtype=np.float32)
    Wv = np.asarray(inputs["Wv"], dtype=np.float32)
    Wo = np.asarray(inputs["Wo"], dtype=np.float32)
    bo = np.asarray(inputs["bo"], dtype=np.float32)

    nc = _get_nc()
    in_maps = _make_in_maps(x, Wq, Wk, Wv, Wo)
    res = run_bass_kernel_spmd(nc, in_maps, list(range(NCORES)), trace=trace)
    out = np.empty((B, S, E), dtype=np.float32)
    for b in range(B):
        out[b] = res.results[2 * b]["out"] + res.results[2 * b + 1]["out"] + bo
    return out, res


def kernel(**inputs):
    out, _ = _run(inputs, trace=False)
    return out


# revision 3
# speedup vs baseline: 1.0620x; 1.0093x over previous
"""Trainium2 Bass kernel v2 for causal MultiHeadAttention (B=4,S=2048,E=1024,H=16).

Sharding: 8 cores = (batch, head-half) grid as v1. Core c: batch c//2, heads
[8*(c%2), 8*(c%2)+8). Host sums the two half-projections per batch + bias.

v2 changes vs v1 (all bf16 — fp8 fails the 2e-2 gate, measured 2.8-6e-2):
  - Scores via PE row-group concurrency: Q/K kept packed [128 = 2 heads x 64dh]
    (no zero-padding, no split copyouts). Per t-tile the two heads' K=64
    matmuls alternate base partitions 0/64 -> concurrent in the PE array
    (measured 73.3ns per [128,256] matmul vs 229ns sequential).
  - CH=256 q-chunks: tighter causal coverage (56.25% vs 62.5%) -> less exp
    and PV work; exp batched over 2 t-tile pairs [128,1024] per head.
  - Chunk-major unit order with a work queue: P2 projection chains, PV
    chains, and P4 out-projection chains are drained into the ACT-throttled
    P3 stream, so the PE never idles between phases and P4 has no tail.
"""

import sys

if "/opt/trn_rl_repo" not in sys.path:
    sys.path.insert(0, "/opt/trn_rl_repo")

import numpy as np
from collections import deque
from contextlib import ExitStack

B, S, E, H = 4, 2048, 1024, 16
DH = E // H          # 64
NCORES = 8
NH = 8               # local heads per core
HP = NH // 2         # head pairs
P = 128
NE = E // P          # 8 e-tiles
NT = S // P          # 16 t-tiles
CH = 256             # q-chunk width
NCH = S // CH        # 8 chunks
SC = S // 512        # 4 sq-supertiles for P2
SCALE = 1.0 / 8.0    # 1/sqrt(DH)

_CACHE = {}


def _build_nc():
    import concourse.mybir as mybir
    import concourse.tile as tile
    import concourse.bass as bass
    from concourse import bacc

    f32 = mybir.dt.float32
    bf16 = mybir.dt.bfloat16
    Exp = mybir.ActivationFunctionType.Exp
    PSUM = bass.MemorySpace.PSUM

    nc = bacc.Bacc(None)
    x_d = nc.dram_tensor("x", [E, S], bf16, kind="ExternalInput")  # pre-transposed
    wq_d = nc.dram_tensor("wq", [E, NH * DH], bf16, kind="ExternalInput")
    wk_d = nc.dram_tensor("wk", [E, NH * DH], bf16, kind="ExternalInput")
    wv_d = nc.dram_tensor("wv", [E, NH * DH], bf16, kind="ExternalInput")
    wo_d = nc.dram_tensor("wo", [NH * DH, E], bf16, kind="ExternalInput")
    mask_d = nc.dram_tensor("mask", [P, 2 * CH], bf16, kind="ExternalInput")
    out_d = nc.dram_tensor("out", [S, E], f32, kind="ExternalOutput")

    with ExitStack() as ctx:
        tc = ctx.enter_context(tile.TileContext(nc))
        persist = ctx.enter_context(tc.tile_pool(name="persist", bufs=1))
        # packed per-head-pair layouts: partitions 0:64 head 2hp, 64:128 head
        # 2hp+1 (dh dim); scores use quadrant matmuls straight off these.
        qt = persist.tile([P, HP, S], bf16)
        kt = persist.tile([P, HP, S], bf16)
        # V natural per (t-tile, head): [64 v-cols | ones | 63 garbage]
        vf = persist.tile([P, NT, NH, P], bf16)
        outT = persist.tile([P, HP, S], bf16)
        wo = persist.tile([P, HP, E], bf16)
        msk = persist.tile([P, 2 * CH], bf16)

        p2 = ctx.enter_context(tc.tile_pool(name="p2", bufs=1))
        xt = p2.tile([P, NE, S], bf16)
        wqs = p2.tile([P, NE, NH * DH], bf16)
        wks = p2.tile([P, NE, NH * DH], bf16)
        wvs = p2.tile([P, NE, NH * DH], bf16)

        # DMAs: x/wv on the SP queue (feeds the first chains); wq/wk/mask/wo
        # on the ACT queue in parallel.
        # Spread the startup-critical loads (x chunk 0 + wq) across four DMA
        # queues so the first QK chains can start ~5us in; wv/wk follow on
        # the same queue pairs, then the rest of x.
        for sc in range(SC):
            cs = slice(sc * 512, (sc + 1) * 512)
            for et in range(NE):
                nc.sync.dma_start(out=xt[:, et, cs],
                                  in_=x_d[et * P:(et + 1) * P, cs])
            if sc == 0:
                for et in range(NE):
                    nc.gpsimd.dma_start(out=wvs[:, et, :],
                                        in_=wv_d[et * P:(et + 1) * P, :])
        for et in range(NE):
            nc.scalar.dma_start(out=wqs[:, et, :], in_=wq_d[et * P:(et + 1) * P, :])
            nc.scalar.dma_start(out=wks[:, et, :], in_=wk_d[et * P:(et + 1) * P, :])
        nc.scalar.dma_start(out=msk, in_=mask_d[:])
        for hp in range(HP):
            nc.scalar.dma_start(out=wo[:, hp, :],
                                in_=wo_d[hp * P:(hp + 1) * P, :])
        nc.vector.memset(vf[:, :, :, DH:DH + 1], 1.0)

        prj = ctx.enter_context(tc.tile_pool(name="prj", bufs=2, space=PSUM))
        scp = ctx.enter_context(tc.tile_pool(name="scp", bufs=2, space=PSUM))
        pvp = ctx.enter_context(tc.tile_pool(name="pvp", bufs=2, space=PSUM))
        ptp = ctx.enter_context(tc.tile_pool(name="ptp", bufs=16))
        pop = ctx.enter_context(tc.tile_pool(name="pop", bufs=10))
        dnp = ctx.enter_context(tc.tile_pool(name="dnp", bufs=8))
        dcp = ctx.enter_context(tc.tile_pool(name="dcp", bufs=2))
        bcp = ctx.enter_context(tc.tile_pool(name="bcp", bufs=6))
        obp = ctx.enter_context(tc.tile_pool(name="obp", bufs=3))
        drp = ctx.enter_context(tc.tile_pool(name="drp", bufs=2, space="DRAM"))

        # ---------- emit helpers ----------
        def emit_v_chain(st):
            ps = prj.tile([P, 512], f32, tag="prj", name="ps")
            for et in range(NE):
                nc.tensor.matmul(ps, xt[:, et, st * P:(st + 1) * P],
                                 wvs[:, et, :], start=(et == 0),
                                 stop=(et == NE - 1), skip_group_check=True)
            nc.vector.tensor_copy(
                out=vf[:, st, :, 0:DH],
                in_=ps.rearrange("p (h d) -> p h d", h=NH))

        def emit_qk_chain(hp, wi, sc):
            w = wqs if wi == 0 else wks
            dst = qt if wi == 0 else kt
            ps = prj.tile([P, 512], f32, tag="prj", name="ps")
            cs = slice(sc * 512, (sc + 1) * 512)
            for et in range(NE):
                nc.tensor.matmul(ps, w[:, et, hp * P:(hp + 1) * P],
                                 xt[:, et, cs], start=(et == 0),
                                 stop=(et == NE - 1), skip_group_check=True)
            nc.vector.tensor_copy(out=dst[:, hp, cs], in_=ps)

        workq = deque()
        reserve = {"on": False}  # hold back P4 items as tail filler

        def drain_budget(ns):
            skipped = deque()
            while workq and ns > 0:
                it = workq.popleft()
                if (reserve["on"] and it["kind"] == "p4"
                        and len(skipped) < 10):
                    skipped.append(it)
                    continue
                it["fn"]()
                ns -= it["cost"]
            workq.extendleft(reversed(skipped))

        def drain_pred(pred):
            keep = deque()
            while workq:
                it = workq.popleft()
                if pred(it):
                    it["fn"]()
                else:
                    keep.append(it)
            workq.extend(keep)

        # ---------- P3 scores unit ----------
        # One burst = up to 4 t-tiles x 2 heads (A/B alternating per t-tile
        # so head-B LDWEIGHTS overlap head-A matmuls), emitted back-to-back
        # with NO intervening queue work; queue drains happen only between
        # bursts, after the exps are emitted.
        def emit_unit(chk, hp):
            ntv = 2 * (chk + 1)
            cs = slice(chk * CH, (chk + 1) * CH)
            pts = {0: [], 1: []}
            for g in range((ntv + 3) // 4):
                tts = list(range(4 * g, min(4 * g + 4, ntv)))
                sps = {0: scp.tile([P, 4, CH], f32, tag="sp", name="sp"),
                       1: scp.tile([P, 4, CH], f32, tag="sp", name="sp")}
                for tt in tts:
                    ts = slice(tt * P, (tt + 1) * P)
                    for h in range(2):
                        b = 64 * h
                        nc.tensor.matmul(
                            sps[h][:, tt - 4 * g, :], kt[b:b + 64, hp, ts],
                            qt[b:b + 64, hp, cs], start=True, stop=True)
                n = len(tts)
                for h in range(2):
                    pt = ptp.tile([P, 4, CH], bf16, tag="pt", name="pt")
                    nc.scalar.activation(
                        out=pt[:, 0:n, :].rearrange("p a b -> p (a b)"),
                        in_=sps[h][:, 0:n, :].rearrange("p a b -> p (a b)"),
                        func=Exp, scale=SCALE)
                    j0 = 2 * chk - 4 * g  # diagonal pair's local slot
                    if 0 <= j0 < 4:
                        nc.vector.tensor_mul(
                            pt[:, j0:j0 + 2, :].rearrange("p a b -> p (a b)"),
                            pt[:, j0:j0 + 2, :].rearrange("p a b -> p (a b)"),
                            msk)
                    pts[h].append(pt)
                drain_budget(2200)
            return pts

        po_units = {}    # chk -> {hp: po tile}
        dchks = {}       # chk -> dens tile

        def emit_pv(chk, hp, pts):
            ntv = 2 * (chk + 1)
            if chk not in dchks:
                dchks[chk] = dcp.tile([NH, CH], f32, tag="dc", name="dc")
            po = pop.tile([P, CH], bf16, tag="po", name="po")
            for h in range(2):
                hl = 2 * hp + h
                pv = pvp.tile([P, 512], f32, tag="pv", name="pv")
                for tt in range(ntv):
                    nc.tensor.matmul(
                        pv[:, 0:CH], vf[:, tt, hl, :],
                        pts[h][tt // 4][:, tt % 4, :],
                        start=(tt == 0), stop=(tt == ntv - 1),
                        skip_group_check=True)
                nc.vector.tensor_copy(out=po[h * DH:(h + 1) * DH, :],
                                      in_=pv[0:DH, 0:CH])
                den = dnp.tile([1, CH], f32, tag="den", name="den")
                nc.vector.tensor_copy(out=den, in_=pv[DH:DH + 1, 0:CH])
                nc.sync.dma_start(
                    out=dchks[chk][2 * hp + h:2 * hp + h + 1, :], in_=den)
            po_units.setdefault(chk, {})[hp] = po

        def emit_finalize(chk):
            dens = dchks.pop(chk)
            for k in range(2):
                ks = slice(k * P, (k + 1) * P)
                nc.vector.reciprocal(out=dens[:, ks], in_=dens[:, ks])
            dd = drp.tile([NH, CH], f32, tag="dd", name="dd")
            nc.sync.dma_start(out=dd, in_=dens)
            pos = po_units.pop(chk)
            cs = slice(chk * CH, (chk + 1) * CH)
            for hp in range(HP):
                bc = bcp.tile([P, CH], f32, tag="bc", name="bc")
                for h in range(2):
                    row = dd[2 * hp + h:2 * hp + h + 1, :]
                    src = bass.AP(tensor=row.tensor, offset=row.offset,
                                  ap=[[0, DH]] + list(row.ap[1:]))
                    nc.sync.dma_start(out=bc[h * DH:(h + 1) * DH, :], in_=src)
                nc.gpsimd.tensor_mul(outT[:, hp, cs], pos[hp], bc)

        def emit_p4(chk, st, ech):
            ps = prj.tile([P, 512], f32, tag="prj", name="ps")
            es = slice(ech * 512, (ech + 1) * 512)
            for hp in range(HP):
                nc.tensor.matmul(ps, outT[:, hp, st * P:(st + 1) * P],
                                 wo[:, hp, es], start=(hp == 0),
                                 stop=(hp == HP - 1), skip_group_check=True)
            ob = obp.tile([P, 512], f32, tag="ob", name="ob")
            nc.vector.tensor_copy(out=ob, in_=ps)
            nc.sync.dma_start(out=out_d[st * P:(st + 1) * P, es], in_=ob)

        # ---------- emission schedule ----------
        # P2 sc=0 inline: QK first (unblocks chunk-0 scores), then V st 0-3.
        for hp in range(HP):
            for wi in range(2):
                emit_qk_chain(hp, wi, 0)
        for st in range(4):
            emit_v_chain(st)
        # queue the rest of P2
        for sc in range(1, SC):
            for hp in range(HP):
                for wi in range(2):
                    workq.append({"kind": "p2", "key": sc, "cost": 2200,
                                  "fn": (lambda hp=hp, wi=wi, sc=sc:
                                         emit_qk_chain(hp, wi, sc))})
            for st in range(4 * sc, 4 * sc + 4):
                workq.append({"kind": "p2", "key": sc, "cost": 2200,
                              "fn": (lambda st=st: emit_v_chain(st))})

        fin_pending = None
        prev_unit = None
        for pos, chk in enumerate(range(NCH)):
            if pos >= 6:
                reserve["on"] = True
            screq = (2 * chk + 1) // 4
            drain_pred(lambda it, r=screq: it["kind"] == "p2" and it["key"] <= r)
            for hp in range(HP):
                pts = emit_unit(chk, hp)
                if prev_unit is not None:
                    emit_pv(*prev_unit)     # lag-1: its exps are long done
                prev_unit = (chk, hp, pts)
                if fin_pending is not None:
                    fc = fin_pending
                    fin_pending = None
                    emit_finalize(fc)
                    for st in (2 * fc, 2 * fc + 1):
                        for ech in range(2):
                            workq.append({"kind": "p4", "key": fc, "cost": 1100,
                                          "fn": (lambda fc=fc, st=st, ech=ech:
                                                 emit_p4(fc, st, ech))})
            fin_pending = chk
        # tail: spend the reserved P4 work first (it fills the PE while ACT
        # chews through the last chunk's exp backlog), then the final
        # PV/finalize/P4 cascade.
        reserve["on"] = False
        drain_pred(lambda it: it["kind"] == "p4")
        emit_pv(*prev_unit)
        drain_pred(lambda it: it["kind"] != "p4")
        emit_finalize(fin_pending)
        for st in (2 * fin_pending, 2 * fin_pending + 1):
            for ech in range(2):
                workq.append({"kind": "p4", "key": fin_pending, "cost": 1100,
                              "fn": (lambda fc=fin_pending, st=st, ech=ech:
                                     emit_p4(fc, st, ech))})
        drain_pred(lambda it: True)

    nc.finalize()
    return nc


def _get_nc():
    if "nc" not in _CACHE:
        _CACHE["nc"] = _build_nc()
    return _CACHE["nc"]


def _make_in_maps(x, Wq, Wk, Wv, Wo):
    import ml_dtypes

    bf = ml_dtypes.bfloat16
    # mask[p, i*CH + n] = 1 iff 128*i + p <= n  (diagonal t-tile pair of any
    # CH=256 chunk; t_local = 128*i + p, sq_local = n)
    pcol = np.arange(P)[:, None]
    nrow = np.arange(CH)[None, :]
    mask = np.concatenate([(pcol <= nrow), (pcol + 128 <= nrow)],
                          axis=1).astype(bf)
    in_maps = []
    for c in range(NCORES):
        b, half = divmod(c, 2)
        hs = slice(half * NH, (half + 1) * NH)
        in_maps.append({
            "x": np.ascontiguousarray(x[b].T.astype(bf)),
            "wq": np.ascontiguousarray(
                Wq[hs].transpose(1, 0, 2).reshape(E, NH * DH).astype(bf)),
            "wk": np.ascontiguousarray(
                Wk[hs].transpose(1, 0, 2).reshape(E, NH * DH).astype(bf)),
            "wv": np.ascontiguousarray(
                Wv[hs].transpose(1, 0, 2).reshape(E, NH * DH).astype(bf)),
            "wo": np.ascontiguousarray(
                Wo[half * NH * DH:(half + 1) * NH * DH].astype(bf)),
            "mask": mask,
        })
    return in_maps


def _ensure_ntff_hook():
    import types
    try:
        import antenv.axon_hooks  # noqa: F401
        return
    except ImportError:
        pass
    try:
        from trn_agent_boot.trn_boot import _ntff_profile_via_ctypes
        hook = _ntff_profile_via_ctypes("/opt/axon/libaxon_pjrt.so")
    except Exception:
        hook = None
    mod = types.ModuleType("antenv.axon_hooks")
    mod.get_axon_ntff_profile_hook = lambda: hook
    mod.set_axon_ntff_profile_hook = lambda h: None
    sys.modules["antenv.axon_hooks"] = mod


def _run(inputs, trace=False):
    from concourse.bass_utils import run_bass_kernel_spmd

    if trace:
        _ensure_ntff_hook()

    x = np.asarray(inputs["x"], dtype=np.float32)
    Wq = np.asarray(inputs["Wq"], dtype=np.float32)
    Wk = np.asarray(inputs["Wk"], dtype=np.float32)
    Wv = np.asarray(inputs["Wv"], dtype=np.float32)
    Wo = np.asarray(inputs["Wo"], dtype=np.float32)
    bo = np.asarray(inputs["bo"], dtype=np.float32)

    nc = _get_nc()
    in_maps = _make_in_maps(x, Wq, Wk, Wv, Wo)
    res = run_bass_kernel_spmd(nc, in_maps, list(range(NCORES)), trace=trace)
    out = np.empty((B, S, E), dtype=np.float32)
    for b in range(B):
        out[b] = res.results[2 * b]["out"] + res.results[2 * b + 1]["out"] + bo
    return out, res


def kernel(**inputs):
    out, _ = _run(inputs, trace=False)
    return out
